# revision 28
# baseline (speedup 1.0000x reference)
"""Trainium2 Bass kernel for the DanceDynamicsModel Lindblad solver.

Full inputs in, full outputs out. Internally:
  - host (numpy): build the 128x128 Hamiltonian H, the 49 Lindblad
    operators L_k, and M = sum_k L_k^T L_k from the tiny MLP inputs.
  - device (8 NeuronCores): the linear Lindblad map F applied to rho0,
    with the 49 L-sandwich terms sharded over cores (k-sharding per the
    hint) and the Hamiltonian/anticommutator terms sharded column-wise
    (16 output columns per core). Each core emits a raw f32 partial;
    the 8-way sum + strip assembly + Taylor combine happen host-side
    (the unshard).

The J=1 fast path exploits rho0 Hermitian-splitting: the device stage
only ever sees a symmetric (or antisymmetric) real matrix G, so the
state is a single 128x128 P (no imaginary half):
    S_c(G)  = sum_{k in core c} L_k G L_k^T          (PSUM accumulate)
    Y_blk   = G @ Bn[:, blk_c]     (Bn = -M/2)       (16-col strip)
    Z_blk   = G @ A[:, blk_c]      (A = H)           (16-col strip)
Host:  S = sum_c S_c,  Y/Z assembled from strips, then
    sym G=P:    Fr = S + Y + Y^T,   Fi = Z - Z^T
    antisym G=K (strips come out negated since lhsT^T = -K):
                Fr = Z_m + Z_m^T,   Fi = S - Y_m + Y_m^T
and rho(tau_t) = rho0 + tau_t * (Fr + i Fi).

All device matmuls are bf16 (PSUM accumulates fp32); validated at
~1e-6 global relative error vs the complex64 reference.

A general RK4/Taylor fallback graph (from the earlier revision) is kept
for inputs where the J=1 Taylor truncation would not pass the host-side
convergence check.
"""
import os
import sys
os.environ.setdefault("JAX_PLATFORMS", "axon,cpu")
for _p in ('/opt/trn_rl_repo',):
    if _p not in sys.path:
        sys.path.insert(0, _p)

import numpy as np
import ml_dtypes

import concourse.bass as bass
import concourse.bacc as bacc
import concourse.tile as tile
import concourse.mybir as mybir

NQ = 7          # qubits ("dancers")
D = 128         # 2**NQ
NCORES = 8
SLOTS = 7       # Lindblad-op slots per core (49 ops, zero-padded to 56)
BLK = D // NCORES  # 16: per-core column strip of the Hamiltonian terms
# ops per core for the legacy path; core 0 also owns the AB terms there
OP_SPLIT = [4, 7, 7, 7, 6, 6, 6, 6]
# ops per core for the fast path (AB terms are column-sharded instead)
OP_SPLIT_FAST = [7, 6, 6, 6, 6, 6, 6, 6]
BF16 = mybir.dt.bfloat16
F32 = mybir.dt.float32
AluOp = mybir.AluOpType


# ----------------------------------------------------------------- host math
def _embed(op, sites):
    k = len(sites)
    full = np.kron(op, np.eye(2 ** (NQ - k), dtype=op.dtype))
    t = full.reshape((2,) * (2 * NQ))
    order = list(sites) + [q for q in range(NQ) if q not in sites]
    inv = np.argsort(np.array(order))
    perm = [int(p) for p in inv] + [NQ + int(p) for p in inv]
    return t.transpose(perm).reshape(D, D)


def _build_operators(features, W1, b1, W2, b2, H_self, H_coupling, rates):
    f32 = np.float32
    h = np.maximum(np.asarray(features, f32) @ np.asarray(W1, f32) + np.asarray(b1, f32), 0)
    ops = (h @ np.asarray(W2, f32) + np.asarray(b2, f32)).reshape(NQ, 2, 2)
    Hs = np.asarray(H_self, f32)
    Hc = np.asarray(H_coupling, f32)
    rates = np.asarray(rates, f32)

    H = np.zeros((D, D), f32)
    for i in range(NQ):
        Hi = ops[i] @ Hs[i] + Hs[i].T @ ops[i].T
        H += _embed(Hi, [i])
    for i in range(NQ):
        for j in range(i + 1, NQ):
            oij = np.kron(ops[i], ops[j])
            Hij = oij @ Hc[i, j] + Hc[i, j].T @ oij.T
            H += _embed(Hij, [i, j])

    Ls = []
    for i in range(NQ):
        for j in range(NQ):
            g = np.sqrt(np.abs(rates[i, j])).astype(f32)
            if i == j:
                Ls.append(_embed(g[:2, :2] * ops[i], [i]))
            else:
                Ls.append(_embed(g * np.kron(ops[i], ops[j]), [i, j]))
    L = np.stack(Ls)                                      # (49, D, D) real
    M = np.einsum('kji,kjl->il', L, L, optimize=True)     # sum_k L^T L
    return H, L, M


# ----------------------------------------------------- fast J=1 device graph
def _build_fast(n_loop=0, body=1, ablate=(), filler=0):
    """One SPMD graph for all 8 cores computing the J=1 partial
    [S_c(G) | G@Bn_blk | G@A_blk] (128 x 160 f32) from G (128 x 128 f32).

    n_loop=0: the real solve - a single stage, partial DMA'd out.
    n_loop>0: timing graph - a For_i hardware loop of `body` chained
      stages per iteration (body must be even so tile-pool buffer phases
      realign at the loop back-edge). Each stage performs the identical
      instruction sequence as the real solve (same matmuls, copies, and
      partial DMA-out) plus one scalar_tensor_tensor that rebuilds the
      next stage's input from the previous stage's output
      (G_next = 0 * out + G0), enforcing a full serial dependency
      between consecutive stages while keeping values finite.
    """
    nc = bacc.Bacc(None, target_bir_lowering=False, debug=False,
                   num_devices=NCORES)
    lt_in = nc.dram_tensor("lt", [D, SLOTS * D], BF16, kind="ExternalInput")
    ab_in = nc.dram_tensor("ab", [D, 2 * BLK], BF16, kind="ExternalInput")
    x0_in = nc.dram_tensor("x0", [D, D], F32, kind="ExternalInput")
    traj = nc.dram_tensor("traj", [D, D + 2 * BLK], F32, kind="ExternalOutput")
    OW = D + 2 * BLK  # 160

    with tile.TileContext(nc) as tc:
        with (
            tc.tile_pool(name="const", bufs=1) as const,
            tc.tile_pool(name="xb", bufs=2) as xp,
            tc.tile_pool(name="vsa", bufs=2) as vsap,
            tc.tile_pool(name="vsb", bufs=2) as vsbp,
            tc.tile_pool(name="vsc", bufs=2) as vscp,
            tc.tile_pool(name="part", bufs=3) as pp,
            tc.tile_pool(name="vpsa", bufs=2, space="PSUM") as vpsa,
            tc.tile_pool(name="vpsb", bufs=2, space="PSUM") as vpsb,
            tc.tile_pool(name="vpsc", bufs=2, space="PSUM") as vpsc,
            tc.tile_pool(name="fps", bufs=2, space="PSUM") as fps,
            tc.tile_pool(name="dout", bufs=4, space="DRAM") as dop,
        ):
            # combined const layout [L_0^T .. L_6^T | Bn_blk A_blk]: the
            # strips matmul rides inside the third (small) V matmul (same
            # stationary Pb), removing one PE matmul + weight load from the
            # critical-path prefix.
            CW = SLOTS * D + 2 * BLK          # 928
            CLT = const.tile([D, CW], BF16, name="CLT")
            x0sb = const.tile([D, D], F32, name="x0sb")
            nc.sync.dma_start(CLT[:, 0:SLOTS * D], lt_in[:])
            nc.sync.dma_start(CLT[:, SLOTS * D:CW], ab_in[:])
            nc.sync.dma_start(x0sb[:], x0_in[:])

            p0 = xp.tile([D, D], BF16, name="p0", tag="xb")
            nc.vector.tensor_copy(p0[:], x0sb[:])

            def stage(it, Pb, chain):
                # V = G @ [L_1^T .. L_7^T] plus the strips, via three PE
                # matmuls sharing the stationary Pb (strips ride inside the
                # first one). The PSUM->SBUF copy of V is unavoidable (PE
                # cannot read PSUM, GPSIMD/Pool cannot access PSUM on trn2);
                # it is split DVE/ACT. Crucially each copy engine gets its
                # OWN psum and sbuf tile: a shared tile would make the tile
                # framework chain the two readers/writers behind one counting
                # semaphore, serializing the copies (~0.5us/stage).
                # Measured-best chunk layout: slots 0-2 on DVE, 3-5 on ACT,
                # slot 6 riding second on DVE (early gate ends the sandwich).
                Vaps = vpsa.tile([D, 384], F32, name=f"vaps{it}", tag="va")
                Vbps = vpsb.tile([D, 384], F32, name=f"vbps{it}", tag="vb")
                Vcps = vpsc.tile([D, 160], F32, name=f"vcps{it}", tag="vc")
                X2 = fps.tile([D, D], F32, name=f"x2{it}", tag="x2")
                Va = vsap.tile([D, 384], BF16, name=f"va{it}", tag="vsa")
                Vb = vsbp.tile([D, 384], BF16, name=f"vb{it}", tag="vsb")
                Vc = vscp.tile([D, 128], BF16, name=f"vc{it}", tag="vsc")
                nc.tensor.matmul(Vaps[:], lhsT=Pb, rhs=CLT[:, 0:384])
                nc.tensor.matmul(Vbps[:], lhsT=Pb, rhs=CLT[:, 384:768])
                nc.tensor.matmul(Vcps[:], lhsT=Pb, rhs=CLT[:, 768:928])
                nc.vector.tensor_copy(Va[:], Vaps[:])
                nc.scalar.copy(Vb[:], Vbps[:])
                nc.vector.tensor_copy(Vc[:], Vcps[:, 0:128])
                part = None
                if "nodma" not in ablate:
                    part = pp.tile([D, OW], F32, name=f"pt{it}", tag="part")
                    # strips land in part[:, D:] straight from the V3 psum
                    # (ACT, early, off the critical path -- keeps the DVE
                    # queue clear ahead of the serializer; emitted before the
                    # serializer so the part write-chain resolves early)
                    nc.scalar.copy(part[:, D:OW], Vcps[:, 128:160])
                # sandwich: S_c = sum_s (V_s)^T @ L_s^T
                for s in range(SLOTS):
                    if s < 3:
                        lhs = Va[:, s * D:(s + 1) * D]
                    elif s < 6:
                        lhs = Vb[:, (s - 3) * D:(s - 2) * D]
                    else:
                        lhs = Vc[:]
                    sl = slice(s * D, (s + 1) * D)
                    nc.tensor.matmul(X2[:], lhsT=lhs, rhs=CLT[:, sl],
                                     start=(s == 0), stop=(s == SLOTS - 1))
                # The chained-timing serializer (next stage's input from
                # this stage's output) is emitted BEFORE the part copy: both
                # read X2, and the tile framework chains same-tile readers in
                # program order -- the off-critical-path reader must go last.
                Pn = None
                if chain and "nostt" in ablate:
                    Pn = p0   # diagnostic only: breaks the serial chain
                elif chain:
                    Pn = xp.tile([D, D], BF16, name=f"pn{it}", tag="xb")
                    nc.vector.scalar_tensor_tensor(Pn[:], X2[:], 0.0,
                                                   x0sb[:], op0=AluOp.mult,
                                                   op1=AluOp.add)
                if "nodma" not in ablate:
                    nc.scalar.copy(part[:, 0:D], X2[:])
                    if chain:
                        # rotate the per-stage output across DRAM buffers:
                        # writing one fixed address every stage would WAW-
                        # serialize the DMAs (~1.4us each on HW), a stall the
                        # one-shot real solve does not have
                        dst = dop.tile([D, OW], F32, name=f"do{it}", tag="do")
                        nc.sync.dma_start(dst[:], part[:])
                    else:
                        nc.sync.dma_start(traj[:, :], part[:])
                return Pn

            if n_loop == 0:
                stage(0, p0, chain=False)
            else:
                assert body % 2 == 0, "body must be even for pool phase"
                with tc.For_i(0, n_loop):
                    Pb = p0
                    for k in range(body):
                        Pb = stage(k, Pb, chain=True)
                # one final un-chained stage emits the graph's real output
                stage(body, p0, chain=False)
    nc.compile()
    return nc


def _in_maps_fast(H, L, M, G):
    bf = ml_dtypes.bfloat16
    Bn = (-0.5 * M).astype(np.float32)
    A = np.asarray(H, np.float32)
    maps, k0 = [], 0
    for c in range(NCORES):
        n = OP_SPLIT_FAST[c]
        lt = np.zeros((D, SLOTS * D), np.float32)
        for s in range(n):
            lt[:, s * D:(s + 1) * D] = L[k0 + s].T
        k0 += n
        blk = slice(c * BLK, (c + 1) * BLK)
        ab = np.concatenate([Bn[:, blk], A[:, blk]], axis=1)
        maps.append({
            "lt": lt.astype(bf),
            "ab": ab.astype(bf),
            "x0": np.asarray(G, np.float32),
        })
    return maps


def _apply_F_fast(runner, H, L, M, G, antisym):
    """Runs the device stage on real G (in the real slot of rho; Q=0) and
    returns (Fr, Fi) = F(G + 0i):
        Fr = Bn G + G Bn + sum_k L G L^T,   Fi = G A - A G.
    The device computes S = sum_k L G L^T plus the strips
    Y_m = G^T Bn, Z_m = G^T A; the transpose is resolved host-side using
    G's (anti)symmetry: Bn G = Y_m^T always, and G Bn = +-Y_m."""
    res = runner.run(_in_maps_fast(H, L, M, G))["traj"]   # [8, 128, 160]
    S = res[:, :, 0:D].sum(axis=0)
    Y = np.concatenate([res[c, :, D:D + BLK] for c in range(NCORES)], axis=1)
    Z = np.concatenate([res[c, :, D + BLK:] for c in range(NCORES)], axis=1)
    if not antisym:
        return S + Y + Y.T, Z - Z.T
    return S + Y.T - Y, -(Z + Z.T)


def _solve_fast(runner, H, L, M, rho0, dts, sym):
    nsteps = len(dts)
    if sym:
        Fr, Fi = _apply_F_fast(runner, H, L, M, rho0, antisym=False)
    else:
        S0 = 0.5 * (rho0 + rho0.T)
        K0 = 0.5 * (rho0 - rho0.T)
        Fr1, Fi1 = _apply_F_fast(runner, H, L, M, S0, antisym=False)
        Fr2, Fi2 = _apply_F_fast(runner, H, L, M, K0, antisym=True)
        Fr, Fi = Fr1 + Fr2, Fi1 + Fi2
    out = np.empty((nsteps + 1, D, D), np.complex64)
    out[0] = rho0
    taus = np.cumsum(np.asarray(dts, np.float64))
    for t in range(nsteps):
        tau = np.float32(taus[t])
        out[t + 1] = (rho0 + tau * Fr) + 1j * (tau * Fi)
    return out


# ------------------------------------------- legacy general device graph
def _build_nc(dts, repeat=1, strategy="ar", taylor_J=None, partial_out=False):
    """General RK4/Taylor graph (kept as fallback; see earlier revision)."""
    nsteps = len(dts)
    nc = bacc.Bacc(None, target_bir_lowering=False, debug=False,
                   num_devices=NCORES)
    lt_in = nc.dram_tensor("lt", [D, SLOTS * D], BF16, kind="ExternalInput")
    ab_in = nc.dram_tensor("ab", [D, 4 * D], BF16, kind="ExternalInput")
    x0_in = nc.dram_tensor("x0", [D, 2 * D], F32, kind="ExternalInput")
    if partial_out:
        traj = nc.dram_tensor("traj", [D, 2 * D], F32, kind="ExternalOutput")
    else:
        traj = nc.dram_tensor("traj", [nsteps, D, 2 * D], F32,
                              kind="ExternalOutput")
    rg = [list(range(NCORES))]

    with tile.TileContext(nc) as tc:
        with (
            tc.tile_pool(name="const", bufs=1) as const,
            tc.tile_pool(name="state", bufs=1) as state,
            tc.tile_pool(name="xb", bufs=2) as xbp,
            tc.tile_pool(name="vsb", bufs=1) as vsb,
            tc.tile_pool(name="pack", bufs=2) as packp,
            tc.tile_pool(name="vps", bufs=1, space="PSUM") as vps,
            tc.tile_pool(name="accps", bufs=1, space="PSUM") as accps,
            tc.tile_pool(name="dram", bufs=2, space="DRAM") as dram,
        ):
            LT = const.tile([D, SLOTS * D], BF16, name="LT")
            AB = const.tile([D, 4 * D], BF16, name="AB")
            nc.sync.dma_start(LT[:], lt_in[:])
            nc.sync.dma_start(AB[:], ab_in[:])

            acc = state.tile([D, 2 * D], F32, name="acc")
            nc.sync.dma_start(acc[:], x0_in[:])

            xb0 = xbp.tile([D, 2 * D], BF16, name="xb0", tag="xb")
            nc.vector.tensor_copy(xb0[:], acc[:])
            Xb = xb0

            def f_stage(it, j, Xb, emit_partial=None):
                P = Xb[:, 0:D]
                Q = Xb[:, D:2 * D]
                A = AB[:, 0:D]
                Bn = AB[:, D:2 * D]
                An = AB[:, 2 * D:3 * D]     # -A
                Bnn = AB[:, 3 * D:4 * D]    # -Bn

                Vp = vps.tile([D, SLOTS * D], F32, name=f"vp{it}_{j}", tag="vp")
                Vq = vps.tile([D, SLOTS * D], F32, name=f"vq{it}_{j}", tag="vq")
                Fr = accps.tile([D, D], F32, name=f"fr{it}_{j}", tag="fr")
                Fip = accps.tile([D, D], F32, name=f"fip{it}_{j}", tag="fip")

                nc.tensor.matmul(Vp[:, 0:512], lhsT=P, rhs=LT[:, 0:512])
                nc.tensor.matmul(Vp[:, 512:896], lhsT=P, rhs=LT[:, 512:896])
                nc.tensor.matmul(Fr[:], lhsT=P, rhs=Bn, start=True, stop=False)
                nc.tensor.matmul(Fip[:], lhsT=P, rhs=A, start=True, stop=False)
                nc.tensor.matmul(Vq[:, 0:512], lhsT=Q, rhs=LT[:, 0:512])
                nc.tensor.matmul(Vq[:, 512:896], lhsT=Q, rhs=LT[:, 512:896])
                nc.tensor.matmul(Fr[:], lhsT=Q, rhs=A, start=False, stop=False)
                nc.tensor.matmul(Fip[:], lhsT=Q, rhs=Bnn, start=False, stop=False)
                nc.tensor.matmul(Fr[:], lhsT=A, rhs=Q, start=False, stop=False)
                nc.tensor.matmul(Fip[:], lhsT=An, rhs=P, start=False, stop=False)
                nc.tensor.matmul(Fr[:], lhsT=Bn, rhs=P, start=False, stop=False)
                nc.tensor.matmul(Fip[:], lhsT=Bn, rhs=Q, start=False, stop=False)

                Vp_sb = vsb.tile([D, SLOTS * D], BF16, name=f"vps{it}_{j}", tag="vpsb")
                Vq_sb = vsb.tile([D, SLOTS * D], BF16, name=f"vqs{it}_{j}", tag="vqsb")
                nc.vector.tensor_copy(Vp_sb[:, 0:512], Vp[:, 0:512])
                nc.vector.tensor_copy(Vp_sb[:, 512:896], Vp[:, 512:896])
                nc.vector.tensor_copy(Vq_sb[:, 0:512], Vq[:, 0:512])
                nc.vector.tensor_copy(Vq_sb[:, 512:896], Vq[:, 512:896])

                for s in range(SLOTS):
                    sl = slice(s * D, (s + 1) * D)
                    nc.tensor.matmul(Fr[:], lhsT=Vp_sb[:, sl], rhs=LT[:, sl],
                                     start=False, stop=(s == SLOTS - 1))
                    nc.tensor.matmul(Fip[:], lhsT=Vq_sb[:, sl], rhs=LT[:, sl],
                                     start=False, stop=(s == SLOTS - 1))

                pdt = F32 if emit_partial is not None else BF16
                part = packp.tile([D, 2 * D], pdt, name=f"pt{it}_{j}", tag="part")
                nc.vector.tensor_copy(part[:, 0:D], Fr[:])
                nc.vector.tensor_copy(part[:, D:2 * D], Fip[:])
                if emit_partial is not None:
                    nc.sync.dma_start(emit_partial, part[:])
                    return None

                cin = dram.tile([D, 2 * D], BF16, name=f"ci{it}_{j}", tag="cin")
                nc.sync.dma_start(cin[:], part[:])
                Xn = xbp.tile([D, 2 * D], BF16, name=f"xb{it}_{j}", tag="xb")
                cout = dram.tile([D, 2 * D], BF16,
                                 name=f"co{it}_{j}", tag="cout")
                nc.gpsimd.collective_compute(
                    "AllReduce", AluOp.add, replica_groups=rg,
                    ins=[cin[:].opt()], outs=[cout[:].opt()])
                nc.sync.dma_start(Xn[:], cout[:])
                return Xn

            if partial_out:
                assert taylor_J == 1
                for rrep in range(repeat):
                    f_stage(rrep, 1, Xb, emit_partial=traj[:, :])
            elif taylor_J is not None:
                import math as _math
                taus = [float(sum(dts[:tt + 1])) for tt in range(nsteps)]
                accs = []
                for tt in range(nsteps):
                    a = state.tile([D, 2 * D], F32, name=f"acc{tt}")
                    nc.sync.dma_start(a[:], x0_in[:])
                    accs.append(a)
                for rrep in range(repeat):
                    Xc = Xb
                    for j in range(1, taylor_J + 1):
                        Xc = f_stage(rrep, j, Xc)
                        for tt in range(nsteps):
                            c = taus[tt] ** j / _math.factorial(j)
                            nc.vector.scalar_tensor_tensor(
                                accs[tt][:], Xc[:], c, accs[tt][:],
                                op0=AluOp.mult, op1=AluOp.add)
                for tt in range(nsteps):
                    nc.sync.dma_start(traj[tt, :, :], accs[tt][:])
            else:
                for it, t in enumerate(
                        [s for _ in range(repeat) for s in range(nsteps)]):
                    dt = float(dts[t])
                    cs = [dt, dt * dt / 2.0, dt ** 3 / 6.0, dt ** 4 / 24.0]
                    for j in range(4):
                        Xn = f_stage(it, j, Xb)
                        nc.vector.scalar_tensor_tensor(
                            acc[:], Xn[:], cs[j], acc[:],
                            op0=AluOp.mult, op1=AluOp.add)
                        Xb = Xn
                    nc.sync.dma_start(traj[t, :, :], acc[:])
                    if it + 1 < nsteps * repeat:
                        xs = xbp.tile([D, 2 * D], BF16, name=f"xs{it}", tag="xb")
                        nc.vector.tensor_copy(xs[:], acc[:])
                        Xb = xs
    nc.compile()
    return nc


# ---------------------------------------------------------------- jit runner
class _Runner:
    """Persistent jitted shard_map executor for a compiled Bass graph
    (mirrors bass2jax.run_bass_via_pjrt, but reusable for timing)."""

    def __init__(self, nc):
        import jax
        from jax.sharding import Mesh, PartitionSpec
        from jax.experimental.shard_map import shard_map
        from concourse import bass2jax
        bass2jax.install_neuronx_cc_hook()

        self.nc = nc
        part_name = nc.partition_id_tensor.name if nc.partition_id_tensor else None
        in_names, out_names, out_avals, zero_outs = [], [], [], []
        for alloc in nc.m.functions[0].allocations:
            if not isinstance(alloc, mybir.MemoryLocationSet):
                continue
            name = alloc.memorylocations[0].name
            if alloc.kind == "ExternalInput":
                if name != part_name:
                    in_names.append(name)
            elif alloc.kind == "ExternalOutput":
                out_names.append(name)
                shape = tuple(alloc.tensor_shape)
                dtype = mybir.dt.np(alloc.dtype)
                out_avals.append(jax.core.ShapedArray(shape, dtype))
                zero_outs.append(np.zeros(shape, dtype))
        self.in_names, self.out_names = in_names, out_names
        self.out_avals, self.zero_outs = out_avals, zero_outs
        n_params, n_outs = len(in_names), len(out_names)

        def _body(*args):
            operands = list(args)
            bind_names = in_names + out_names
            if part_name is not None:
                operands.append(bass2jax.partition_id_tensor())
                bind_names = bind_names + [part_name]
            outs = bass2jax._bass_exec_p.bind(
                *operands,
                out_avals=tuple(out_avals),
                in_names=tuple(bind_names),
                out_names=tuple(out_names),
                lowering_input_output_aliases=(),
                sim_require_finite=True,
                sim_require_nnan=True,
                nc=nc,
            )
            return tuple(outs)

        devices = jax.devices()[:NCORES]
        self.mesh = Mesh(np.asarray(devices), ("core",))
        specs = (PartitionSpec("core"),) * (n_params + n_outs)
        self.fn = jax.jit(
            shard_map(_body, mesh=self.mesh, in_specs=specs,
                      out_specs=(PartitionSpec("core"),) * n_outs,
                      check_rep=False),
            donate_argnums=tuple(range(n_params, n_params + n_outs)),
            keep_unused=True,
        )
        self.jax = jax

    def _concat_inputs(self, in_maps):
        return [np.concatenate([np.asarray(in_maps[c][n]) for c in range(NCORES)],
                               axis=0) for n in self.in_names]

    def _zeros(self):
        return [np.zeros((NCORES * z.shape[0], *z.shape[1:]), z.dtype)
                for z in self.zero_outs]

    def run(self, in_maps):
        outs = self.fn(*self._concat_inputs(in_maps), *self._zeros())
        return {
            n: np.asarray(outs[i]).reshape(NCORES, *self.out_avals[i].shape)
            for i, n in enumerate(self.out_names)
        }

    def time(self, in_maps, reps=30):
        """Median-of-batches pipelined timing: returns est seconds/execution."""
        import time as _time
        cin = [self.jax.device_put(x) for x in self._concat_inputs(in_maps)]
        zsets = [[self.jax.device_put(z) for z in self._zeros()]
                 for _ in range(reps)]
        self.jax.block_until_ready((cin, zsets))
        outs = self.fn(*cin, *zsets[0])          # warm
        self.jax.block_until_ready(outs)
        t0 = _time.time()
        res = [self.fn(*cin, *z) for z in zsets[1:]]
        self.jax.block_until_ready(res)
        t1 = _time.time()
        return (t1 - t0) / max(1, reps - 1)


# -------------------------------------------------------------------- driver
_CACHE = {}


def _get_fast_runner(n_loop=0, body=1, ablate=(), filler=0):
    key = ("fast", n_loop, body, tuple(ablate), filler)
    if key not in _CACHE:
        _CACHE[key] = _Runner(_build_fast(n_loop, body, ablate, filler))
    return _CACHE[key]


def _get_runner(dts, taylor_J=None):
    key = (tuple(np.float32(d) for d in dts), taylor_J)
    if key not in _CACHE:
        _CACHE[key] = _Runner(_build_nc([float(d) for d in key[0]],
                                        strategy="ar",
                                        taylor_J=taylor_J,
                                        partial_out=(taylor_J == 1)))
    return _CACHE[key]


def _pick_taylor_J(H, L, M, dts):
    """Host-side convergence check for the direct Taylor evaluation
    rho(tau_t) = sum_j tau_t^j/j! F^j(rho0). Returns J if the series
    converges fast enough AND the reference's RK4 is itself within ~1e-6
    of the exact exponential; else None (use the RK4-replication path)."""
    import math
    A, Bn = H, -0.5 * M
    taus = np.cumsum(np.asarray(dts, np.float64))

    def Fm(P, Q):
        LP = L @ P
        SP = np.einsum('kij,kmj->im', LP, L, optimize=True)
        LQ = L @ Q
        SQ = np.einsum('kij,kmj->im', LQ, L, optimize=True)
        return (A @ Q - Q @ A + Bn @ P + P @ Bn + SP,
                -A @ P + P @ A + Bn @ Q + Q @ Bn + SQ)

    P = np.eye(D, dtype=np.float32)
    Q = np.zeros_like(P)
    tmax = float(taus[-1])
    scale = np.linalg.norm(P)
    wn = [scale]
    for j in range(1, 17):
        P, Q = Fm(P, Q)
        wn.append(float(np.hypot(np.linalg.norm(P), np.linalg.norm(Q))))
        tail = tmax ** j / math.factorial(j) * wn[j]
        if j >= 5 and tail < 1e-8 * scale:
            dt5 = max(float(d) for d in dts) ** 5
            rk4_gap = dt5 / 120.0 * wn[5] * len(dts)
            if rk4_gap < 1e-6 * scale:
                for jj in range(1, j):
                    drop = tmax ** (jj + 1) / math.factorial(jj + 1) * wn[jj + 1]
                    if drop < 1e-6 * scale:
                        return jj
            return None
    return None


def _in_maps(H, L, M, P0, Q0):
    bf = ml_dtypes.bfloat16
    Bn = (-0.5 * M).astype(np.float32)
    ab0 = np.concatenate([H, Bn, -H, -Bn], axis=1).astype(bf)
    x0 = np.concatenate([np.asarray(P0, np.float32),
                         np.asarray(Q0, np.float32)], axis=1)
    maps, k0 = [], 0
    for c in range(NCORES):
        n = OP_SPLIT[c]
        lt = np.zeros((D, SLOTS * D), np.float32)
        for s in range(n):
            lt[:, s * D:(s + 1) * D] = L[k0 + s].T
        k0 += n
        maps.append({
            "lt": lt.astype(bf),
            "ab": ab0 if c == 0 else np.zeros_like(ab0),
            "x0": x0,
        })
    return maps


def _solve(runner, H, L, M, P0, Q0, dts, taylor_J=None):
    nsteps = len(dts)
    res = runner.run(_in_maps(H, L, M, P0, Q0))
    out = np.empty((nsteps + 1, D, D), np.complex64)
    P0 = np.asarray(P0, np.float32)
    Q0 = np.asarray(Q0, np.float32)
    out[0] = P0 + 1j * Q0
    if taylor_J == 1:
        w = res["traj"].sum(axis=0)          # [128, 256] f32
        taus = np.cumsum(np.asarray(dts, np.float64))
        for t in range(nsteps):
            tau = np.float32(taus[t])
            out[t + 1] = (P0 + tau * w[:, 0:D]) + 1j * (Q0 + tau * w[:, D:2 * D])
        return out
    tr = res["traj"][0]          # all cores identical; [nsteps, 128, 256]
    for t in range(nsteps):
        out[t + 1] = tr[t, :, 0:D] + 1j * tr[t, :, D:2 * D]
    return out


def kernel(features, t_eval, W1, b1, W2, b2, H_self, H_coupling,
           lindblad_rates, rho_0):
    H, L, M = _build_operators(features, W1, b1, W2, b2,
                               H_self, H_coupling, lindblad_rates)
    t_eval = np.asarray(t_eval, np.float32)
    dts = (t_eval[1:] - t_eval[:-1]).astype(np.float32)
    taylor_J = _pick_taylor_J(H, L, M, dts)

    rho0 = np.asarray(rho_0, np.float32)
    sym = np.abs(rho0 - rho0.T).max() <= 1e-6 * max(1.0, np.abs(rho0).max())

    if taylor_J == 1:
        runner = _get_fast_runner(0)
        return _solve_fast(runner, H, L, M, rho0, dts, sym)

    runner = _get_runner(dts, taylor_J)
    if sym:
        return _solve(runner, H, L, M, rho0, np.zeros_like(rho0), dts, taylor_J)
    S = 0.5 * (rho0 + rho0.T)
    K = 0.5 * (rho0 - rho0.T)
    tA = _solve(runner, H, L, M, S, np.zeros_like(S), dts, taylor_J)
    tB = _solve(runner, H, L, M, np.zeros_like(K), K, dts, taylor_J)
    return (tA + (-1j) * tB).astype(np.complex64)


# revision 29
# speedup vs baseline: 1.0047x; 1.0047x over previous
"""Trainium2 Bass kernel for the DanceDynamicsModel Lindblad solver.

Full inputs in, full outputs out. Internally:
  - host (numpy): build the 128x128 Hamiltonian H, the 49 Lindblad
    operators L_k, and M = sum_k L_k^T L_k from the tiny MLP inputs.
  - device (8 NeuronCores): the linear Lindblad map F applied to rho0,
    with the 49 L-sandwich terms sharded over cores (k-sharding per the
    hint) and the Hamiltonian/anticommutator terms sharded column-wise
    (16 output columns per core). Each core emits a raw f32 partial;
    the 8-way sum + strip assembly + Taylor combine happen host-side
    (the unshard).

The J=1 fast path exploits rho0 Hermitian-splitting: the device stage
only ever sees a symmetric (or antisymmetric) real matrix G, so the
state is a single 128x128 P (no imaginary half):
    S_c(G)  = sum_{k in core c} L_k G L_k^T          (PSUM accumulate)
    Y_blk   = G @ Bn[:, blk_c]     (Bn = -M/2)       (16-col strip)
    Z_blk   = G @ A[:, blk_c]      (A = H)           (16-col strip)
Host:  S = sum_c S_c,  Y/Z assembled from strips, then
    sym G=P:    Fr = S + Y + Y^T,   Fi = Z - Z^T
    antisym G=K (strips come out negated since lhsT^T = -K):
                Fr = Z_m + Z_m^T,   Fi = S - Y_m + Y_m^T
and rho(tau_t) = rho0 + tau_t * (Fr + i Fi).

All device matmuls are bf16 (PSUM accumulates fp32); validated at
~1e-6 global relative error vs the complex64 reference.

A general RK4/Taylor fallback graph (from the earlier revision) is kept
for inputs where the J=1 Taylor truncation would not pass the host-side
convergence check.
"""
import os
import sys
os.environ.setdefault("JAX_PLATFORMS", "axon,cpu")
for _p in ('/opt/trn_rl_repo',):
    if _p not in sys.path:
        sys.path.insert(0, _p)

import numpy as np
import ml_dtypes

import concourse.bass as bass
import concourse.bacc as bacc
import concourse.tile as tile
import concourse.mybir as mybir

NQ = 7          # qubits ("dancers")
D = 128         # 2**NQ
NCORES = 8
SLOTS = 7       # Lindblad-op slots per core (49 ops, zero-padded to 56)
BLK = D // NCORES  # 16: per-core column strip of the Hamiltonian terms
# ops per core for the legacy path; core 0 also owns the AB terms there
OP_SPLIT = [4, 7, 7, 7, 6, 6, 6, 6]
# ops per core for the fast path (AB terms are column-sharded instead)
OP_SPLIT_FAST = [7, 6, 6, 6, 6, 6, 6, 6]
BF16 = mybir.dt.bfloat16
F32 = mybir.dt.float32
AluOp = mybir.AluOpType


# ----------------------------------------------------------------- host math
def _embed(op, sites):
    k = len(sites)
    full = np.kron(op, np.eye(2 ** (NQ - k), dtype=op.dtype))
    t = full.reshape((2,) * (2 * NQ))
    order = list(sites) + [q for q in range(NQ) if q not in sites]
    inv = np.argsort(np.array(order))
    perm = [int(p) for p in inv] + [NQ + int(p) for p in inv]
    return t.transpose(perm).reshape(D, D)


def _build_operators(features, W1, b1, W2, b2, H_self, H_coupling, rates):
    f32 = np.float32
    h = np.maximum(np.asarray(features, f32) @ np.asarray(W1, f32) + np.asarray(b1, f32), 0)
    ops = (h @ np.asarray(W2, f32) + np.asarray(b2, f32)).reshape(NQ, 2, 2)
    Hs = np.asarray(H_self, f32)
    Hc = np.asarray(H_coupling, f32)
    rates = np.asarray(rates, f32)

    H = np.zeros((D, D), f32)
    for i in range(NQ):
        Hi = ops[i] @ Hs[i] + Hs[i].T @ ops[i].T
        H += _embed(Hi, [i])
    for i in range(NQ):
        for j in range(i + 1, NQ):
            oij = np.kron(ops[i], ops[j])
            Hij = oij @ Hc[i, j] + Hc[i, j].T @ oij.T
            H += _embed(Hij, [i, j])

    Ls = []
    for i in range(NQ):
        for j in range(NQ):
            g = np.sqrt(np.abs(rates[i, j])).astype(f32)
            if i == j:
                Ls.append(_embed(g[:2, :2] * ops[i], [i]))
            else:
                Ls.append(_embed(g * np.kron(ops[i], ops[j]), [i, j]))
    L = np.stack(Ls)                                      # (49, D, D) real
    M = np.einsum('kji,kjl->il', L, L, optimize=True)     # sum_k L^T L
    return H, L, M


# ----------------------------------------------------- fast J=1 device graph
def _build_fast(n_loop=0, body=1, ablate=(), filler=0):
    """One SPMD graph for all 8 cores computing the J=1 partial
    [S_c(G) | G@Bn_blk | G@A_blk] (128 x 160 f32) from G (128 x 128 f32).

    n_loop=0: the real solve - a single stage, partial DMA'd out.
    n_loop>0: timing graph - a For_i hardware loop of `body` chained
      stages per iteration (body must be even so tile-pool buffer phases
      realign at the loop back-edge). Each stage performs the identical
      instruction sequence as the real solve (same matmuls, copies, and
      partial DMA-out) plus one scalar_tensor_tensor that rebuilds the
      next stage's input from the previous stage's output
      (G_next = 0 * out + G0), enforcing a full serial dependency
      between consecutive stages while keeping values finite.
    """
    nc = bacc.Bacc(None, target_bir_lowering=False, debug=False,
                   num_devices=NCORES)
    lt_in = nc.dram_tensor("lt", [D, SLOTS * D], BF16, kind="ExternalInput")
    ab_in = nc.dram_tensor("ab", [D, 2 * BLK], BF16, kind="ExternalInput")
    x0_in = nc.dram_tensor("x0", [D, D], F32, kind="ExternalInput")
    traj = nc.dram_tensor("traj", [D, D + 2 * BLK], F32, kind="ExternalOutput")
    OW = D + 2 * BLK  # 160

    with tile.TileContext(nc) as tc:
        with (
            tc.tile_pool(name="const", bufs=1) as const,
            tc.tile_pool(name="xb", bufs=2) as xp,
            tc.tile_pool(name="vsa", bufs=2) as vsap,
            tc.tile_pool(name="vsb", bufs=2) as vsbp,
            tc.tile_pool(name="vsc", bufs=2) as vscp,
            tc.tile_pool(name="part", bufs=3) as pp,
            tc.tile_pool(name="vpsa", bufs=2, space="PSUM") as vpsa,
            tc.tile_pool(name="vpsb", bufs=2, space="PSUM") as vpsb,
            tc.tile_pool(name="vpsc", bufs=2, space="PSUM") as vpsc,
            tc.tile_pool(name="fps", bufs=2, space="PSUM") as fps,
            tc.tile_pool(name="dout", bufs=4, space="DRAM") as dop,
        ):
            # combined const layout [L_0^T .. L_6^T | Bn_blk A_blk]: the
            # strips matmul rides inside the third (small) V matmul (same
            # stationary Pb), removing one PE matmul + weight load from the
            # critical-path prefix.
            CW = SLOTS * D + 2 * BLK          # 928
            CLT = const.tile([D, CW], BF16, name="CLT")
            x0sb = const.tile([D, D], F32, name="x0sb")
            nc.sync.dma_start(CLT[:, 0:SLOTS * D], lt_in[:])
            nc.sync.dma_start(CLT[:, SLOTS * D:CW], ab_in[:])
            nc.sync.dma_start(x0sb[:], x0_in[:])

            p0 = xp.tile([D, D], BF16, name="p0", tag="xb")
            nc.vector.tensor_copy(p0[:], x0sb[:])

            def stage(it, Pb, chain):
                # V = G @ [L_1^T .. L_7^T] plus the strips, via three PE
                # matmuls sharing the stationary Pb (strips ride inside the
                # first one). The PSUM->SBUF copy of V is unavoidable (PE
                # cannot read PSUM, GPSIMD/Pool cannot access PSUM on trn2);
                # it is split DVE/ACT. Crucially each copy engine gets its
                # OWN psum and sbuf tile: a shared tile would make the tile
                # framework chain the two readers/writers behind one counting
                # semaphore, serializing the copies (~0.5us/stage).
                # Measured-best chunk layout: slots 0-2 on DVE, 3-5 on ACT,
                # slot 6 riding second on DVE (early gate ends the sandwich).
                Vaps = vpsa.tile([D, 384], F32, name=f"vaps{it}", tag="va")
                Vbps = vpsb.tile([D, 384], F32, name=f"vbps{it}", tag="vb")
                Vcps = vpsc.tile([D, 160], F32, name=f"vcps{it}", tag="vc")
                X2 = fps.tile([D, D], F32, name=f"x2{it}", tag="x2")
                Va = vsap.tile([D, 384], BF16, name=f"va{it}", tag="vsa")
                Vb = vsbp.tile([D, 384], BF16, name=f"vb{it}", tag="vsb")
                Vc = vscp.tile([D, 128], BF16, name=f"vc{it}", tag="vsc")
                nc.tensor.matmul(Vaps[:], lhsT=Pb, rhs=CLT[:, 0:384])
                nc.tensor.matmul(Vbps[:], lhsT=Pb, rhs=CLT[:, 384:768])
                nc.tensor.matmul(Vcps[:], lhsT=Pb, rhs=CLT[:, 768:928])
                nc.vector.tensor_copy(Va[:], Vaps[:])
                nc.scalar.copy(Vb[:], Vbps[:])
                nc.vector.tensor_copy(Vc[:], Vcps[:, 0:128])
                part = None
                if "nodma" not in ablate:
                    part = pp.tile([D, OW], F32, name=f"pt{it}", tag="part")
                    # strips land in part[:, D:] straight from the V3 psum
                    # (ACT, early, off the critical path -- keeps the DVE
                    # queue clear ahead of the serializer; emitted before the
                    # serializer so the part write-chain resolves early)
                    nc.scalar.copy(part[:, D:OW], Vcps[:, 128:160])
                # sandwich: S_c = sum_s (V_s)^T @ L_s^T
                for s in range(SLOTS):
                    if s < 3:
                        lhs = Va[:, s * D:(s + 1) * D]
                    elif s < 6:
                        lhs = Vb[:, (s - 3) * D:(s - 2) * D]
                    else:
                        lhs = Vc[:]
                    sl = slice(s * D, (s + 1) * D)
                    nc.tensor.matmul(X2[:], lhsT=lhs, rhs=CLT[:, sl],
                                     start=(s == 0), stop=(s == SLOTS - 1))
                # The chained-timing serializer (next stage's input from
                # this stage's output) is emitted BEFORE the part copy: both
                # read X2, and the tile framework chains same-tile readers in
                # program order -- the off-critical-path reader must go last.
                Pn = None
                if chain and "nostt" in ablate:
                    Pn = p0   # diagnostic only: breaks the serial chain
                elif chain and "actser" in ablate:
                    # serializer on ACT: Pn = Copy(X2 * 0.0) -- same full
                    # data dependency, zero-valued chain input
                    Pn = xp.tile([D, D], BF16, name=f"pn{it}", tag="xb")
                    nc.scalar.mul(Pn[:], X2[:], 0.0)
                elif chain:
                    Pn = xp.tile([D, D], BF16, name=f"pn{it}", tag="xb")
                    nc.vector.scalar_tensor_tensor(Pn[:], X2[:], 0.0,
                                                   x0sb[:], op0=AluOp.mult,
                                                   op1=AluOp.add)
                if "nodma" not in ablate:
                    nc.scalar.copy(part[:, 0:D], X2[:])
                    if chain:
                        # rotate the per-stage output across DRAM buffers:
                        # writing one fixed address every stage would WAW-
                        # serialize the DMAs (~1.4us each on HW), a stall the
                        # one-shot real solve does not have
                        dst = dop.tile([D, OW], F32, name=f"do{it}", tag="do")
                        nc.sync.dma_start(dst[:], part[:])
                    else:
                        nc.sync.dma_start(traj[:, :], part[:])
                return Pn

            if n_loop == 0:
                stage(0, p0, chain=False)
            else:
                assert body % 2 == 0, "body must be even for pool phase"
                with tc.For_i(0, n_loop):
                    Pb = p0
                    for k in range(body):
                        Pb = stage(k, Pb, chain=True)
                # one final un-chained stage emits the graph's real output
                stage(body, p0, chain=False)
    nc.compile()
    return nc


def _in_maps_fast(H, L, M, G):
    bf = ml_dtypes.bfloat16
    Bn = (-0.5 * M).astype(np.float32)
    A = np.asarray(H, np.float32)
    maps, k0 = [], 0
    for c in range(NCORES):
        n = OP_SPLIT_FAST[c]
        lt = np.zeros((D, SLOTS * D), np.float32)
        for s in range(n):
            lt[:, s * D:(s + 1) * D] = L[k0 + s].T
        k0 += n
        blk = slice(c * BLK, (c + 1) * BLK)
        ab = np.concatenate([Bn[:, blk], A[:, blk]], axis=1)
        maps.append({
            "lt": lt.astype(bf),
            "ab": ab.astype(bf),
            "x0": np.asarray(G, np.float32),
        })
    return maps


def _apply_F_fast(runner, H, L, M, G, antisym):
    """Runs the device stage on real G (in the real slot of rho; Q=0) and
    returns (Fr, Fi) = F(G + 0i):
        Fr = Bn G + G Bn + sum_k L G L^T,   Fi = G A - A G.
    The device computes S = sum_k L G L^T plus the strips
    Y_m = G^T Bn, Z_m = G^T A; the transpose is resolved host-side using
    G's (anti)symmetry: Bn G = Y_m^T always, and G Bn = +-Y_m."""
    res = runner.run(_in_maps_fast(H, L, M, G))["traj"]   # [8, 128, 160]
    S = res[:, :, 0:D].sum(axis=0)
    Y = np.concatenate([res[c, :, D:D + BLK] for c in range(NCORES)], axis=1)
    Z = np.concatenate([res[c, :, D + BLK:] for c in range(NCORES)], axis=1)
    if not antisym:
        return S + Y + Y.T, Z - Z.T
    return S + Y.T - Y, -(Z + Z.T)


def _solve_fast(runner, H, L, M, rho0, dts, sym):
    nsteps = len(dts)
    if sym:
        Fr, Fi = _apply_F_fast(runner, H, L, M, rho0, antisym=False)
    else:
        S0 = 0.5 * (rho0 + rho0.T)
        K0 = 0.5 * (rho0 - rho0.T)
        Fr1, Fi1 = _apply_F_fast(runner, H, L, M, S0, antisym=False)
        Fr2, Fi2 = _apply_F_fast(runner, H, L, M, K0, antisym=True)
        Fr, Fi = Fr1 + Fr2, Fi1 + Fi2
    out = np.empty((nsteps + 1, D, D), np.complex64)
    out[0] = rho0
    taus = np.cumsum(np.asarray(dts, np.float64))
    for t in range(nsteps):
        tau = np.float32(taus[t])
        out[t + 1] = (rho0 + tau * Fr) + 1j * (tau * Fi)
    return out


# ------------------------------------------- legacy general device graph
def _build_nc(dts, repeat=1, strategy="ar", taylor_J=None, partial_out=False):
    """General RK4/Taylor graph (kept as fallback; see earlier revision)."""
    nsteps = len(dts)
    nc = bacc.Bacc(None, target_bir_lowering=False, debug=False,
                   num_devices=NCORES)
    lt_in = nc.dram_tensor("lt", [D, SLOTS * D], BF16, kind="ExternalInput")
    ab_in = nc.dram_tensor("ab", [D, 4 * D], BF16, kind="ExternalInput")
    x0_in = nc.dram_tensor("x0", [D, 2 * D], F32, kind="ExternalInput")
    if partial_out:
        traj = nc.dram_tensor("traj", [D, 2 * D], F32, kind="ExternalOutput")
    else:
        traj = nc.dram_tensor("traj", [nsteps, D, 2 * D], F32,
                              kind="ExternalOutput")
    rg = [list(range(NCORES))]

    with tile.TileContext(nc) as tc:
        with (
            tc.tile_pool(name="const", bufs=1) as const,
            tc.tile_pool(name="state", bufs=1) as state,
            tc.tile_pool(name="xb", bufs=2) as xbp,
            tc.tile_pool(name="vsb", bufs=1) as vsb,
            tc.tile_pool(name="pack", bufs=2) as packp,
            tc.tile_pool(name="vps", bufs=1, space="PSUM") as vps,
            tc.tile_pool(name="accps", bufs=1, space="PSUM") as accps,
            tc.tile_pool(name="dram", bufs=2, space="DRAM") as dram,
        ):
            LT = const.tile([D, SLOTS * D], BF16, name="LT")
            AB = const.tile([D, 4 * D], BF16, name="AB")
            nc.sync.dma_start(LT[:], lt_in[:])
            nc.sync.dma_start(AB[:], ab_in[:])

            acc = state.tile([D, 2 * D], F32, name="acc")
            nc.sync.dma_start(acc[:], x0_in[:])

            xb0 = xbp.tile([D, 2 * D], BF16, name="xb0", tag="xb")
            nc.vector.tensor_copy(xb0[:], acc[:])
            Xb = xb0

            def f_stage(it, j, Xb, emit_partial=None):
                P = Xb[:, 0:D]
                Q = Xb[:, D:2 * D]
                A = AB[:, 0:D]
                Bn = AB[:, D:2 * D]
                An = AB[:, 2 * D:3 * D]     # -A
                Bnn = AB[:, 3 * D:4 * D]    # -Bn

                Vp = vps.tile([D, SLOTS * D], F32, name=f"vp{it}_{j}", tag="vp")
                Vq = vps.tile([D, SLOTS * D], F32, name=f"vq{it}_{j}", tag="vq")
                Fr = accps.tile([D, D], F32, name=f"fr{it}_{j}", tag="fr")
                Fip = accps.tile([D, D], F32, name=f"fip{it}_{j}", tag="fip")

                nc.tensor.matmul(Vp[:, 0:512], lhsT=P, rhs=LT[:, 0:512])
                nc.tensor.matmul(Vp[:, 512:896], lhsT=P, rhs=LT[:, 512:896])
                nc.tensor.matmul(Fr[:], lhsT=P, rhs=Bn, start=True, stop=False)
                nc.tensor.matmul(Fip[:], lhsT=P, rhs=A, start=True, stop=False)
                nc.tensor.matmul(Vq[:, 0:512], lhsT=Q, rhs=LT[:, 0:512])
                nc.tensor.matmul(Vq[:, 512:896], lhsT=Q, rhs=LT[:, 512:896])
                nc.tensor.matmul(Fr[:], lhsT=Q, rhs=A, start=False, stop=False)
                nc.tensor.matmul(Fip[:], lhsT=Q, rhs=Bnn, start=False, stop=False)
                nc.tensor.matmul(Fr[:], lhsT=A, rhs=Q, start=False, stop=False)
                nc.tensor.matmul(Fip[:], lhsT=An, rhs=P, start=False, stop=False)
                nc.tensor.matmul(Fr[:], lhsT=Bn, rhs=P, start=False, stop=False)
                nc.tensor.matmul(Fip[:], lhsT=Bn, rhs=Q, start=False, stop=False)

                Vp_sb = vsb.tile([D, SLOTS * D], BF16, name=f"vps{it}_{j}", tag="vpsb")
                Vq_sb = vsb.tile([D, SLOTS * D], BF16, name=f"vqs{it}_{j}", tag="vqsb")
                nc.vector.tensor_copy(Vp_sb[:, 0:512], Vp[:, 0:512])
                nc.vector.tensor_copy(Vp_sb[:, 512:896], Vp[:, 512:896])
                nc.vector.tensor_copy(Vq_sb[:, 0:512], Vq[:, 0:512])
                nc.vector.tensor_copy(Vq_sb[:, 512:896], Vq[:, 512:896])

                for s in range(SLOTS):
                    sl = slice(s * D, (s + 1) * D)
                    nc.tensor.matmul(Fr[:], lhsT=Vp_sb[:, sl], rhs=LT[:, sl],
                                     start=False, stop=(s == SLOTS - 1))
                    nc.tensor.matmul(Fip[:], lhsT=Vq_sb[:, sl], rhs=LT[:, sl],
                                     start=False, stop=(s == SLOTS - 1))

                pdt = F32 if emit_partial is not None else BF16
                part = packp.tile([D, 2 * D], pdt, name=f"pt{it}_{j}", tag="part")
                nc.vector.tensor_copy(part[:, 0:D], Fr[:])
                nc.vector.tensor_copy(part[:, D:2 * D], Fip[:])
                if emit_partial is not None:
                    nc.sync.dma_start(emit_partial, part[:])
                    return None

                cin = dram.tile([D, 2 * D], BF16, name=f"ci{it}_{j}", tag="cin")
                nc.sync.dma_start(cin[:], part[:])
                Xn = xbp.tile([D, 2 * D], BF16, name=f"xb{it}_{j}", tag="xb")
                cout = dram.tile([D, 2 * D], BF16,
                                 name=f"co{it}_{j}", tag="cout")
                nc.gpsimd.collective_compute(
                    "AllReduce", AluOp.add, replica_groups=rg,
                    ins=[cin[:].opt()], outs=[cout[:].opt()])
                nc.sync.dma_start(Xn[:], cout[:])
                return Xn

            if partial_out:
                assert taylor_J == 1
                for rrep in range(repeat):
                    f_stage(rrep, 1, Xb, emit_partial=traj[:, :])
            elif taylor_J is not None:
                import math as _math
                taus = [float(sum(dts[:tt + 1])) for tt in range(nsteps)]
                accs = []
                for tt in range(nsteps):
                    a = state.tile([D, 2 * D], F32, name=f"acc{tt}")
                    nc.sync.dma_start(a[:], x0_in[:])
                    accs.append(a)
                for rrep in range(repeat):
                    Xc = Xb
                    for j in range(1, taylor_J + 1):
                        Xc = f_stage(rrep, j, Xc)
                        for tt in range(nsteps):
                            c = taus[tt] ** j / _math.factorial(j)
                            nc.vector.scalar_tensor_tensor(
                                accs[tt][:], Xc[:], c, accs[tt][:],
                                op0=AluOp.mult, op1=AluOp.add)
                for tt in range(nsteps):
                    nc.sync.dma_start(traj[tt, :, :], accs[tt][:])
            else:
                for it, t in enumerate(
                        [s for _ in range(repeat) for s in range(nsteps)]):
                    dt = float(dts[t])
                    cs = [dt, dt * dt / 2.0, dt ** 3 / 6.0, dt ** 4 / 24.0]
                    for j in range(4):
                        Xn = f_stage(it, j, Xb)
                        nc.vector.scalar_tensor_tensor(
                            acc[:], Xn[:], cs[j], acc[:],
                            op0=AluOp.mult, op1=AluOp.add)
                        Xb = Xn
                    nc.sync.dma_start(traj[t, :, :], acc[:])
                    if it + 1 < nsteps * repeat:
                        xs = xbp.tile([D, 2 * D], BF16, name=f"xs{it}", tag="xb")
                        nc.vector.tensor_copy(xs[:], acc[:])
                        Xb = xs
    nc.compile()
    return nc


# ---------------------------------------------------------------- jit runner
class _Runner:
    """Persistent jitted shard_map executor for a compiled Bass graph
    (mirrors bass2jax.run_bass_via_pjrt, but reusable for timing)."""

    def __init__(self, nc):
        import jax
        from jax.sharding import Mesh, PartitionSpec
        from jax.experimental.shard_map import shard_map
        from concourse import bass2jax
        bass2jax.install_neuronx_cc_hook()

        self.nc = nc
        part_name = nc.partition_id_tensor.name if nc.partition_id_tensor else None
        in_names, out_names, out_avals, zero_outs = [], [], [], []
        for alloc in nc.m.functions[0].allocations:
            if not isinstance(alloc, mybir.MemoryLocationSet):
                continue
            name = alloc.memorylocations[0].name
            if alloc.kind == "ExternalInput":
                if name != part_name:
                    in_names.append(name)
            elif alloc.kind == "ExternalOutput":
                out_names.append(name)
                shape = tuple(alloc.tensor_shape)
                dtype = mybir.dt.np(alloc.dtype)
                out_avals.append(jax.core.ShapedArray(shape, dtype))
                zero_outs.append(np.zeros(shape, dtype))
        self.in_names, self.out_names = in_names, out_names
        self.out_avals, self.zero_outs = out_avals, zero_outs
        n_params, n_outs = len(in_names), len(out_names)

        def _body(*args):
            operands = list(args)
            bind_names = in_names + out_names
            if part_name is not None:
                operands.append(bass2jax.partition_id_tensor())
                bind_names = bind_names + [part_name]
            outs = bass2jax._bass_exec_p.bind(
                *operands,
                out_avals=tuple(out_avals),
                in_names=tuple(bind_names),
                out_names=tuple(out_names),
                lowering_input_output_aliases=(),
                sim_require_finite=True,
                sim_require_nnan=True,
                nc=nc,
            )
            return tuple(outs)

        devices = jax.devices()[:NCORES]
        self.mesh = Mesh(np.asarray(devices), ("core",))
        specs = (PartitionSpec("core"),) * (n_params + n_outs)
        self.fn = jax.jit(
            shard_map(_body, mesh=self.mesh, in_specs=specs,
                      out_specs=(PartitionSpec("core"),) * n_outs,
                      check_rep=False),
            donate_argnums=tuple(range(n_params, n_params + n_outs)),
            keep_unused=True,
        )
        self.jax = jax

    def _concat_inputs(self, in_maps):
        return [np.concatenate([np.asarray(in_maps[c][n]) for c in range(NCORES)],
                               axis=0) for n in self.in_names]

    def _zeros(self):
        return [np.zeros((NCORES * z.shape[0], *z.shape[1:]), z.dtype)
                for z in self.zero_outs]

    def run(self, in_maps):
        outs = self.fn(*self._concat_inputs(in_maps), *self._zeros())
        return {
            n: np.asarray(outs[i]).reshape(NCORES, *self.out_avals[i].shape)
            for i, n in enumerate(self.out_names)
        }

    def time(self, in_maps, reps=30):
        """Median-of-batches pipelined timing: returns est seconds/execution."""
        import time as _time
        cin = [self.jax.device_put(x) for x in self._concat_inputs(in_maps)]
        zsets = [[self.jax.device_put(z) for z in self._zeros()]
                 for _ in range(reps)]
        self.jax.block_until_ready((cin, zsets))
        outs = self.fn(*cin, *zsets[0])          # warm
        self.jax.block_until_ready(outs)
        t0 = _time.time()
        res = [self.fn(*cin, *z) for z in zsets[1:]]
        self.jax.block_until_ready(res)
        t1 = _time.time()
        return (t1 - t0) / max(1, reps - 1)


# -------------------------------------------------------------------- driver
_CACHE = {}


def _get_fast_runner(n_loop=0, body=1, ablate=(), filler=0):
    key = ("fast", n_loop, body, tuple(ablate), filler)
    if key not in _CACHE:
        _CACHE[key] = _Runner(_build_fast(n_loop, body, ablate, filler))
    return _CACHE[key]


def _get_runner(dts, taylor_J=None):
    key = (tuple(np.float32(d) for d in dts), taylor_J)
    if key not in _CACHE:
        _CACHE[key] = _Runner(_build_nc([float(d) for d in key[0]],
                                        strategy="ar",
                                        taylor_J=taylor_J,
                                        partial_out=(taylor_J == 1)))
    return _CACHE[key]


def _pick_taylor_J(H, L, M, dts):
    """Host-side convergence check for the direct Taylor evaluation
    rho(tau_t) = sum_j tau_t^j/j! F^j(rho0). Returns J if the series
    converges fast enough AND the reference's RK4 is itself within ~1e-6
    of the exact exponential; else None (use the RK4-replication path)."""
    import math
    A, Bn = H, -0.5 * M
    taus = np.cumsum(np.asarray(dts, np.float64))

    def Fm(P, Q):
        LP = L @ P
        SP = np.einsum('kij,kmj->im', LP, L, optimize=True)
        LQ = L @ Q
        SQ = np.einsum('kij,kmj->im', LQ, L, optimize=True)
        return (A @ Q - Q @ A + Bn @ P + P @ Bn + SP,
                -A @ P + P @ A + Bn @ Q + Q @ Bn + SQ)

    P = np.eye(D, dtype=np.float32)
    Q = np.zeros_like(P)
    tmax = float(taus[-1])
    scale = np.linalg.norm(P)
    wn = [scale]
    for j in range(1, 17):
        P, Q = Fm(P, Q)
        wn.append(float(np.hypot(np.linalg.norm(P), np.linalg.norm(Q))))
        tail = tmax ** j / math.factorial(j) * wn[j]
        if j >= 5 and tail < 1e-8 * scale:
            dt5 = max(float(d) for d in dts) ** 5
            rk4_gap = dt5 / 120.0 * wn[5] * len(dts)
            if rk4_gap < 1e-6 * scale:
                for jj in range(1, j):
                    drop = tmax ** (jj + 1) / math.factorial(jj + 1) * wn[jj + 1]
                    if drop < 1e-6 * scale:
                        return jj
            return None
    return None


def _in_maps(H, L, M, P0, Q0):
    bf = ml_dtypes.bfloat16
    Bn = (-0.5 * M).astype(np.float32)
    ab0 = np.concatenate([H, Bn, -H, -Bn], axis=1).astype(bf)
    x0 = np.concatenate([np.asarray(P0, np.float32),
                         np.asarray(Q0, np.float32)], axis=1)
    maps, k0 = [], 0
    for c in range(NCORES):
        n = OP_SPLIT[c]
        lt = np.zeros((D, SLOTS * D), np.float32)
        for s in range(n):
            lt[:, s * D:(s + 1) * D] = L[k0 + s].T
        k0 += n
        maps.append({
            "lt": lt.astype(bf),
            "ab": ab0 if c == 0 else np.zeros_like(ab0),
            "x0": x0,
        })
    return maps


def _solve(runner, H, L, M, P0, Q0, dts, taylor_J=None):
    nsteps = len(dts)
    res = runner.run(_in_maps(H, L, M, P0, Q0))
    out = np.empty((nsteps + 1, D, D), np.complex64)
    P0 = np.asarray(P0, np.float32)
    Q0 = np.asarray(Q0, np.float32)
    out[0] = P0 + 1j * Q0
    if taylor_J == 1:
        w = res["traj"].sum(axis=0)          # [128, 256] f32
        taus = np.cumsum(np.asarray(dts, np.float64))
        for t in range(nsteps):
            tau = np.float32(taus[t])
            out[t + 1] = (P0 + tau * w[:, 0:D]) + 1j * (Q0 + tau * w[:, D:2 * D])
        return out
    tr = res["traj"][0]          # all cores identical; [nsteps, 128, 256]
    for t in range(nsteps):
        out[t + 1] = tr[t, :, 0:D] + 1j * tr[t, :, D:2 * D]
    return out


def kernel(features, t_eval, W1, b1, W2, b2, H_self, H_coupling,
           lindblad_rates, rho_0):
    H, L, M = _build_operators(features, W1, b1, W2, b2,
                               H_self, H_coupling, lindblad_rates)
    t_eval = np.asarray(t_eval, np.float32)
    dts = (t_eval[1:] - t_eval[:-1]).astype(np.float32)
    taylor_J = _pick_taylor_J(H, L, M, dts)

    rho0 = np.asarray(rho_0, np.float32)
    sym = np.abs(rho0 - rho0.T).max() <= 1e-6 * max(1.0, np.abs(rho0).max())

    if taylor_J == 1:
        runner = _get_fast_runner(0)
        return _solve_fast(runner, H, L, M, rho0, dts, sym)

    runner = _get_runner(dts, taylor_J)
    if sym:
        return _solve(runner, H, L, M, rho0, np.zeros_like(rho0), dts, taylor_J)
    S = 0.5 * (rho0 + rho0.T)
    K = 0.5 * (rho0 - rho0.T)
    tA = _solve(runner, H, L, M, S, np.zeros_like(S), dts, taylor_J)
    tB = _solve(runner, H, L, M, np.zeros_like(K), K, dts, taylor_J)
    return (tA + (-1j) * tB).astype(np.complex64)
